# revision 24
# baseline (speedup 1.0000x reference)
# BitLinear (eval path) Trainium2 kernel: ternary weight quant + int8 activation
# quant + dense matmul, tensor-parallel over 8 NeuronCores.
#
# Math (per reference):
#   w_scale[o] = max(mean_k |W[o,k]|, EPS)
#   w_quant    = clip(round(W / w_scale), -1, 1)            (ternary)
#   x_scale[t] = max(max_k |x[t,k]| / 127, EPS)
#   x_quant    = round(x / x_scale)                          (int8 range)
#   out[t,o]   = (sum_k x_quant[t,k] * w_quant[o,k]) * x_scale[t] * w_scale[o] + bias[o]
#
# Exactness: w_quant exact in fp8e4/bf16, x_quant (|v|<=127) exact in bf16,
# partials exact in fp32 PSUM (<= 127*4096 < 2^24). Magic-constant rounding
# (v + 1.5*2^23 rounds to rint(v) in the f32 add) and a Relu chain give the
# ternary clip entirely on the scalar engine.
#
# Layout strategy: both X and W are quantized in NATURAL row-major layout
# ([tokens, K] / [out_features, K]) where the per-row scales are
# per-PARTITION: the amax/abs-sum reduces are contiguous single DVE ops and
# the scale multiply + magic round fuse into one scalar-engine pass
# (func(in*scale + bias) with a [128,1] scale). The quantized bf16 rows are
# then transposed into the K-major matmul layout with the DMA xbar transpose
# (one dma_start_transpose per 128-row group). No GpSimd broadcasts, no
# partition reductions, W is read from HBM only once, and the host does no
# transposes at all. Matmuls are token-major: bf16 x_quant stationary, fp8
# weights moving 512 wide, fp32 PSUM, epilogue on DVE + GpSimd.
import numpy as np

import concourse.bacc as bacc
import concourse.bass as bass
import concourse.tile as tile
from concourse import mybir
from concourse.bass_utils import run_bass_kernel_spmd
from concourse.masks import make_identity

F32 = mybir.dt.float32
BF16 = mybir.dt.bfloat16
FP8 = mybir.dt.float8e4

EPS = 1e-5
MAGIC = 12582912.0  # 1.5 * 2^23

B, S, I, O = 4, 2048, 4096, 4096
T_FULL = B * S
TSPLIT, OSPLIT = 4, 2
N_CORES = TSPLIT * OSPLIT

A = mybir.AluOpType
AF = mybir.ActivationFunctionType


def build_nc(K=I, TO=O // OSPLIT, TT=T_FULL // TSPLIT):
    """Per-core program: x [TT, K], w [TO, K], bias [TO] -> out [TT, TO]."""
    KT = K // 128      # 32 k subtiles
    GT = 128           # tokens / out-rows per group
    NG = TT // GT      # 16 token groups
    NB = TO // GT      # 16 W blocks
    OC = 512           # moving width per matmul
    NOC = TO // OC     # 4 o-chunks

    nc = bacc.Bacc("TRN2", target_bir_lowering=False, debug=False)
    x_d = nc.dram_tensor("x", [TT, K], F32, kind="ExternalInput").ap()
    w_d = nc.dram_tensor("w", [TO, K], F32, kind="ExternalInput").ap()
    bias_d = nc.dram_tensor("bias", [TO], F32, kind="ExternalInput").ap()
    out_d = nc.dram_tensor("out", [TT, TO], F32, kind="ExternalOutput").ap()

    with tile.TileContext(nc) as tc:
        with (
            tc.tile_pool(name="ld", bufs=4) as p_ld,      # 16KB f32 row-major loads
            tc.tile_pool(name="bt", bufs=2) as p_bt,      # 8KB bf16 quantized rows
            tc.tile_pool(name="wst", bufs=1) as p_wst,    # 8KB transposed W staging
            tc.tile_pool(name="wq", bufs=1) as p_wq,      # resident fp8 weights
            tc.tile_pool(name="xq", bufs=5) as p_xq,      # bf16 K-major token tiles
            tc.tile_pool(name="sml", bufs=2) as p_sml,
            tc.tile_pool(name="osb", bufs=2) as p_osb,
            tc.tile_pool(name="const", bufs=1) as p_const,
            tc.tile_pool(name="ps_mm", bufs=5, space="PSUM") as ps_mm,
            tc.tile_pool(name="ps_tr", bufs=2, space="PSUM") as ps_tr,
        ):
            ident = p_const.tile([128, 128], F32)
            make_identity(nc, ident[:])
            mag_col = p_const.tile([128, 1], F32)
            nc.vector.memset(mag_col[:], MAGIC)
            nmag_col = p_const.tile([128, 1], F32)
            nc.vector.memset(nmag_col[:], -MAGIC)
            nmag1_col = p_const.tile([128, 1], F32)
            nc.vector.memset(nmag1_col[:], -(MAGIC - 1.0))
            two_col = p_const.tile([128, 1], F32)
            nc.vector.memset(two_col[:], 2.0)
            one_col = p_const.tile([128, 1], F32)
            nc.vector.memset(one_col[:], 1.0)
            xs_cols = p_const.tile([128, NG], F32)    # x_scale, t on partitions
            ws_epi = p_const.tile([128, TO], BF16)    # w_scale bcast rows
            bias_bc = p_const.tile([128, TO], BF16)   # bias bcast rows
            nc.gpsimd.dma_start(
                out=bias_bc[:],
                in_=bass.AP(
                    tensor=bias_d.tensor, offset=bias_d.offset,
                    ap=[[0, 128], [1, TO]],
                ),
            )

            wq_oc = [
                p_wq.tile([128, KT, OC], FP8, name=f"wq_{oc}") for oc in range(NOC)
            ]
            xq_tiles = {}

            # ---------- x group: rows in, amax, quantize, xbar transpose ----
            def x_group(tg):
                xg = p_ld.tile([128, K], F32, tag="ld")
                nc.sync.dma_start(out=xg[:], in_=x_d[tg * GT : (tg + 1) * GT, :])
                am = p_sml.tile([128, 1], F32, tag="am")
                nc.vector.tensor_reduce(
                    out=am[:], in_=xg[:], axis=mybir.AxisListType.X,
                    op=A.max, apply_absolute_value=True,
                )
                nc.vector.tensor_scalar(
                    out=xs_cols[:, tg : tg + 1], in0=am[:],
                    scalar1=1.0 / 127.0, scalar2=EPS, op0=A.mult, op1=A.max,
                )
                rxs = p_sml.tile([128, 1], F32, tag="rxs")
                nc.vector.reciprocal(rxs[:], xs_cols[:, tg : tg + 1])
                # u = x*(1/xs) + M : scale multiply and exact rint in one pass
                nc.scalar.activation(
                    out=xg[:], in_=xg[:], func=AF.Identity,
                    scale=rxs[:], bias=mag_col[:],
                )
                xot = p_bt.tile([128, K], BF16, tag="bt")
                nc.scalar.activation(
                    out=xot[:], in_=xg[:], func=AF.Identity, bias=nmag_col[:],
                )
                xq_t = p_xq.tile([128, KT, GT], BF16, tag="xq")
                nc.sync.dma_start_transpose(xq_t[:], xot[:])
                xq_tiles[tg] = xq_t

            # ---------- W block: rows in, scales, ternary, transpose --------
            def w_block(ob):
                wg = p_ld.tile([128, K], F32, tag="ld")
                nc.sync.dma_start(out=wg[:], in_=w_d[ob * GT : (ob + 1) * GT, :])
                wsum = p_sml.tile([128, 1], F32, tag="wsum")
                nc.vector.tensor_reduce(
                    out=wsum[:], in_=wg[:], axis=mybir.AxisListType.X,
                    op=A.add, apply_absolute_value=True,
                )
                wsf = p_sml.tile([128, 1], F32, tag="wsf")
                nc.vector.tensor_scalar(
                    out=wsf[:], in0=wsum[:], scalar1=1.0 / K, scalar2=EPS,
                    op0=A.mult, op1=A.max,
                )
                rws = p_sml.tile([128, 1], F32, tag="rws")
                nc.vector.reciprocal(rws[:], wsf[:])
                # ws column -> broadcast row slice of ws_epi (PE transpose)
                ptr = ps_tr.tile([1, 128], F32, tag="tr")
                nc.tensor.transpose(ptr[:], wsf[:], ident[:])
                wsrow = p_sml.tile([1, 128], BF16, tag="wsrow")
                nc.scalar.copy(wsrow[:], ptr[:])
                nc.gpsimd.partition_broadcast(
                    ws_epi[:, ob * GT : (ob + 1) * GT], wsrow[:]
                )
                # u = w*(1/ws) + M (exact rint in the f32 add, scalar engine),
                # ternary clip in the magic domain (gpsimd), -M + cast (scalar)
                nc.scalar.activation(
                    out=wg[:], in_=wg[:], func=AF.Identity,
                    scale=rws[:], bias=mag_col[:],
                )
                nc.gpsimd.tensor_scalar(
                    out=wg[:], in0=wg[:], scalar1=MAGIC + 1.0,
                    scalar2=MAGIC - 1.0, op0=A.min, op1=A.max,
                )
                wot = p_bt.tile([128, K], BF16, tag="bt")
                nc.scalar.activation(
                    out=wot[:], in_=wg[:], func=AF.Identity, bias=nmag_col[:],
                )
                wstg = p_wst.tile([128, KT, GT], BF16, tag="wst")
                nc.sync.dma_start_transpose(wstg[:], wot[:])
                # cast to fp8 {-1,0,1} into the resident slice
                oc, osl = ob // (OC // GT), (ob % (OC // GT)) * GT
                nc.vector.tensor_copy(
                    wq_oc[oc][:, :, osl : osl + GT], wstg[:]
                )

            # ---------- matmul pass ----------
            def mm_pass(tg, oc):
                xq_t = xq_tiles[tg]
                pm = ps_mm.tile([128, OC], F32, tag="mm")
                for kt in range(KT):
                    nc.tensor.matmul(
                        pm[:],
                        xq_t[:, kt, :],
                        wq_oc[oc][:, kt, :],
                        start=(kt == 0),
                        stop=(kt == KT - 1),
                    )
                osb = p_osb.tile([128, OC], F32, tag="osb")
                nc.vector.scalar_tensor_tensor(
                    out=osb[:], in0=pm[:], scalar=xs_cols[:, tg : tg + 1],
                    in1=ws_epi[:, oc * OC : (oc + 1) * OC], op0=A.mult, op1=A.mult,
                )
                nc.gpsimd.tensor_tensor(
                    out=osb[:], in0=osb[:],
                    in1=bias_bc[:, oc * OC : (oc + 1) * OC], op=A.add,
                )
                nc.sync.dma_start(
                    out=out_d[tg * GT : (tg + 1) * GT, oc * OC : (oc + 1) * OC],
                    in_=osb[:],
                )

            # ---------- main schedule ----------
            # Fill: first token section + W blocks 0-3 (o-chunk 0).
            x_group(0)
            x_group(1)
            w_block(0)
            w_block(1)
            x_group(2)
            w_block(2)
            w_block(3)
            x_group(3)
            wb_next = 4
            for sec in range(4):
                for oc in range(NOC):
                    for tg in range(sec * 4, sec * 4 + 4):
                        mm_pass(tg, oc)
                    # W blocks for oc+1 issued one slot ahead of their readers
                    if sec == 0 and wb_next < NB:
                        for _ in range(4):
                            w_block(wb_next)
                            wb_next += 1
                    # prefetch next section's token groups
                    if sec < 3 and oc in (1, 2):
                        g = sec * 4 + 4 + (oc - 1) * 2
                        x_group(g)
                        x_group(g + 1)
    nc.compile()
    return nc


_NC_CACHE = {}
LAST_EXEC_NS = None


def _get_nc():
    if "full" not in _NC_CACHE:
        _NC_CACHE["full"] = build_nc()
    return _NC_CACHE["full"]


def _run(x, weight, bias, trace=False):
    global LAST_EXEC_NS
    x = np.asarray(x, dtype=np.float32).reshape(T_FULL, I)
    weight = np.asarray(weight, dtype=np.float32)
    bias = np.asarray(bias, dtype=np.float32)

    TT = T_FULL // TSPLIT
    TO = O // OSPLIT
    in_maps = []
    for c in range(N_CORES):
        ti, oj = divmod(c, OSPLIT)
        in_maps.append(
            {
                "x": np.ascontiguousarray(x[ti * TT : (ti + 1) * TT, :]),
                "w": np.ascontiguousarray(weight[oj * TO : (oj + 1) * TO, :]),
                "bias": np.ascontiguousarray(bias[oj * TO : (oj + 1) * TO]),
            }
        )

    nc = _get_nc()
    res = run_bass_kernel_spmd(
        nc, in_maps, core_ids=list(range(N_CORES)), trace=trace
    )
    LAST_EXEC_NS = res.exec_time_ns

    out = np.empty((T_FULL, O), dtype=np.float32)
    for c in range(N_CORES):
        ti, oj = divmod(c, OSPLIT)
        out[ti * TT : (ti + 1) * TT, oj * TO : (oj + 1) * TO] = res.results[c]["out"]
    return out.reshape(B, S, O)


def kernel(x, weight, bias):
    return _run(x, weight, bias, trace=False)


def kernel_traced(x, weight, bias):
    _run(x, weight, bias, trace=True)
    return LAST_EXEC_NS


# revision 25
# speedup vs baseline: 1.0189x; 1.0189x over previous
# BitLinear (eval path) Trainium2 kernel: ternary weight quant + int8 activation
# quant + dense matmul, tensor-parallel over 8 NeuronCores.
#
# Math (per reference):
#   w_scale[o] = max(mean_k |W[o,k]|, EPS)
#   w_quant    = clip(round(W / w_scale), -1, 1)            (ternary)
#   x_scale[t] = max(max_k |x[t,k]| / 127, EPS)
#   x_quant    = round(x / x_scale)                          (int8 range)
#   out[t,o]   = (sum_k x_quant[t,k] * w_quant[o,k]) * x_scale[t] * w_scale[o] + bias[o]
#
# Exactness: w_quant exact in fp8e4/bf16, x_quant (|v|<=127) exact in bf16,
# partials exact in fp32 PSUM (<= 127*4096 < 2^24). Magic-constant rounding
# (v + 1.5*2^23 rounds to rint(v) in the f32 add) and a Relu chain give the
# ternary clip entirely on the scalar engine.
#
# Layout strategy: both X and W are quantized in NATURAL row-major layout
# ([tokens, K] / [out_features, K]) where the per-row scales are
# per-PARTITION: the amax/abs-sum reduces are contiguous single DVE ops and
# the scale multiply + magic round fuse into one scalar-engine pass
# (func(in*scale + bias) with a [128,1] scale). The quantized bf16 rows are
# then transposed into the K-major matmul layout with the DMA xbar transpose
# (one dma_start_transpose per 128-row group). No GpSimd broadcasts, no
# partition reductions, W is read from HBM only once, and the host does no
# transposes at all. Matmuls are token-major: bf16 x_quant stationary, fp8
# weights moving 512 wide, fp32 PSUM, epilogue on DVE + GpSimd.
import numpy as np

import concourse.bacc as bacc
import concourse.bass as bass
import concourse.tile as tile
from concourse import mybir
from concourse.bass_utils import run_bass_kernel_spmd
from concourse.masks import make_identity

F32 = mybir.dt.float32
BF16 = mybir.dt.bfloat16
FP8 = mybir.dt.float8e4

EPS = 1e-5
MAGIC = 12582912.0  # 1.5 * 2^23

B, S, I, O = 4, 2048, 4096, 4096
T_FULL = B * S
TSPLIT, OSPLIT = 4, 2
N_CORES = TSPLIT * OSPLIT

A = mybir.AluOpType
AF = mybir.ActivationFunctionType


def build_nc(K=I, TO=O // OSPLIT, TT=T_FULL // TSPLIT):
    """Per-core program: x [TT, K], w [TO, K], bias [TO] -> out [TT, TO]."""
    KT = K // 128      # 32 k subtiles
    GT = 128           # tokens / out-rows per group
    NG = TT // GT      # 16 token groups
    NB = TO // GT      # 16 W blocks
    OC = 512           # moving width per matmul
    NOC = TO // OC     # 4 o-chunks

    nc = bacc.Bacc("TRN2", target_bir_lowering=False, debug=False)
    x_d = nc.dram_tensor("x", [TT, K], F32, kind="ExternalInput").ap()
    w_d = nc.dram_tensor("w", [TO, K], F32, kind="ExternalInput").ap()
    bias_d = nc.dram_tensor("bias", [TO], F32, kind="ExternalInput").ap()
    out_d = nc.dram_tensor("out", [TT, TO], F32, kind="ExternalOutput").ap()

    with tile.TileContext(nc) as tc:
        with (
            tc.tile_pool(name="lx", bufs=2) as p_lx,      # 16KB f32 x row loads
            tc.tile_pool(name="lw", bufs=2) as p_lw,      # 16KB f32 W row loads
            tc.tile_pool(name="btx", bufs=1) as p_btx,    # 8KB bf16 x rows
            tc.tile_pool(name="btw", bufs=1) as p_btw,    # 8KB bf16 W rows
            tc.tile_pool(name="wst", bufs=1) as p_wst,    # 8KB transposed W staging
            tc.tile_pool(name="wq", bufs=1) as p_wq,      # resident fp8 weights
            tc.tile_pool(name="xq", bufs=5) as p_xq,      # bf16 K-major token tiles
            tc.tile_pool(name="sml", bufs=2) as p_sml,
            tc.tile_pool(name="osb", bufs=2) as p_osb,
            tc.tile_pool(name="const", bufs=1) as p_const,
            tc.tile_pool(name="ps_mm", bufs=5, space="PSUM") as ps_mm,
            tc.tile_pool(name="ps_tr", bufs=2, space="PSUM") as ps_tr,
        ):
            ident = p_const.tile([128, 128], F32)
            make_identity(nc, ident[:])
            mag_col = p_const.tile([128, 1], F32)
            nc.vector.memset(mag_col[:], MAGIC)
            nmag_col = p_const.tile([128, 1], F32)
            nc.vector.memset(nmag_col[:], -MAGIC)
            nmag1_col = p_const.tile([128, 1], F32)
            nc.vector.memset(nmag1_col[:], -(MAGIC - 1.0))
            two_col = p_const.tile([128, 1], F32)
            nc.vector.memset(two_col[:], 2.0)
            one_col = p_const.tile([128, 1], F32)
            nc.vector.memset(one_col[:], 1.0)
            xs_cols = p_const.tile([128, NG], F32)    # x_scale, t on partitions
            ws_epi = p_const.tile([128, TO], BF16)    # w_scale bcast rows
            bias_bc = p_const.tile([128, TO], BF16)   # bias bcast rows
            nc.gpsimd.dma_start(
                out=bias_bc[:],
                in_=bass.AP(
                    tensor=bias_d.tensor, offset=bias_d.offset,
                    ap=[[0, 128], [1, TO]],
                ),
            )

            wq_oc = [
                p_wq.tile([128, KT, OC], FP8, name=f"wq_{oc}") for oc in range(NOC)
            ]
            xq_tiles = {}

            # ---------- x group: rows in, amax, quantize, xbar transpose ----
            def x_group(tg):
                xg = p_lx.tile([128, K], F32, tag="lx")
                nc.sync.dma_start(out=xg[:], in_=x_d[tg * GT : (tg + 1) * GT, :])
                am = p_sml.tile([128, 1], F32, tag="am")
                nc.vector.tensor_reduce(
                    out=am[:], in_=xg[:], axis=mybir.AxisListType.X,
                    op=A.max, apply_absolute_value=True,
                )
                nc.vector.tensor_scalar(
                    out=xs_cols[:, tg : tg + 1], in0=am[:],
                    scalar1=1.0 / 127.0, scalar2=EPS, op0=A.mult, op1=A.max,
                )
                rxs = p_sml.tile([128, 1], F32, tag="rxs")
                nc.vector.reciprocal(rxs[:], xs_cols[:, tg : tg + 1])
                # u = x*(1/xs) + M : scale multiply and exact rint in one pass
                nc.scalar.activation(
                    out=xg[:], in_=xg[:], func=AF.Identity,
                    scale=rxs[:], bias=mag_col[:],
                )
                xot = p_btx.tile([128, K], BF16, tag="btx")
                nc.scalar.activation(
                    out=xot[:], in_=xg[:], func=AF.Identity, bias=nmag_col[:],
                )
                xq_t = p_xq.tile([128, KT, GT], BF16, tag="xq")
                nc.sync.dma_start_transpose(xq_t[:], xot[:])
                xq_tiles[tg] = xq_t

            # ---------- W block: rows in, scales, ternary, transpose --------
            def w_block(ob):
                wg = p_lw.tile([128, K], F32, tag="lw")
                nc.sync.dma_start(out=wg[:], in_=w_d[ob * GT : (ob + 1) * GT, :])
                wsum = p_sml.tile([128, 1], F32, tag="wsum")
                nc.vector.tensor_reduce(
                    out=wsum[:], in_=wg[:], axis=mybir.AxisListType.X,
                    op=A.add, apply_absolute_value=True,
                )
                wsf = p_sml.tile([128, 1], F32, tag="wsf")
                nc.vector.tensor_scalar(
                    out=wsf[:], in0=wsum[:], scalar1=1.0 / K, scalar2=EPS,
                    op0=A.mult, op1=A.max,
                )
                rws = p_sml.tile([128, 1], F32, tag="rws")
                nc.vector.reciprocal(rws[:], wsf[:])
                # ws column -> broadcast row slice of ws_epi (PE transpose)
                ptr = ps_tr.tile([1, 128], F32, tag="tr")
                nc.tensor.transpose(ptr[:], wsf[:], ident[:])
                wsrow = p_sml.tile([1, 128], BF16, tag="wsrow")
                nc.scalar.copy(wsrow[:], ptr[:])
                nc.gpsimd.partition_broadcast(
                    ws_epi[:, ob * GT : (ob + 1) * GT], wsrow[:]
                )
                # u = w*(1/ws) + M (exact rint in the f32 add, scalar engine),
                # ternary clip in the magic domain (gpsimd), -M + cast (scalar)
                nc.scalar.activation(
                    out=wg[:], in_=wg[:], func=AF.Identity,
                    scale=rws[:], bias=mag_col[:],
                )
                nc.gpsimd.tensor_scalar(
                    out=wg[:], in0=wg[:], scalar1=MAGIC + 1.0,
                    scalar2=MAGIC - 1.0, op0=A.min, op1=A.max,
                )
                wot = p_btw.tile([128, K], BF16, tag="btw")
                nc.scalar.activation(
                    out=wot[:], in_=wg[:], func=AF.Identity, bias=nmag_col[:],
                )
                wstg = p_wst.tile([128, KT, GT], BF16, tag="wst")
                nc.sync.dma_start_transpose(wstg[:], wot[:])
                # cast to fp8 {-1,0,1} into the resident slice
                oc, osl = ob // (OC // GT), (ob % (OC // GT)) * GT
                nc.vector.tensor_copy(
                    wq_oc[oc][:, :, osl : osl + GT], wstg[:]
                )

            # ---------- matmul pass ----------
            def mm_pass(tg, oc):
                xq_t = xq_tiles[tg]
                pm = ps_mm.tile([128, OC], F32, tag="mm")
                for kt in range(KT):
                    nc.tensor.matmul(
                        pm[:],
                        xq_t[:, kt, :],
                        wq_oc[oc][:, kt, :],
                        start=(kt == 0),
                        stop=(kt == KT - 1),
                    )
                osb = p_osb.tile([128, OC], F32, tag="osb")
                nc.vector.scalar_tensor_tensor(
                    out=osb[:], in0=pm[:], scalar=xs_cols[:, tg : tg + 1],
                    in1=ws_epi[:, oc * OC : (oc + 1) * OC], op0=A.mult, op1=A.mult,
                )
                nc.gpsimd.tensor_tensor(
                    out=osb[:], in0=osb[:],
                    in1=bias_bc[:, oc * OC : (oc + 1) * OC], op=A.add,
                )
                nc.sync.dma_start(
                    out=out_d[tg * GT : (tg + 1) * GT, oc * OC : (oc + 1) * OC],
                    in_=osb[:],
                )

            # ---------- main schedule ----------
            # Fill: first token section + W blocks 0-3 (o-chunk 0).
            x_group(0)
            x_group(1)
            w_block(0)
            w_block(1)
            x_group(2)
            w_block(2)
            w_block(3)
            x_group(3)
            wb_next = 4
            for sec in range(4):
                for oc in range(NOC):
                    for tg in range(sec * 4, sec * 4 + 4):
                        mm_pass(tg, oc)
                    # prefetch next section's token groups
                    if sec < 3 and oc in (1, 2):
                        g = sec * 4 + 4 + (oc - 1) * 2
                        x_group(g)
                        x_group(g + 1)
                    # W blocks for oc+1 issued one slot ahead of their readers
                    if sec == 0 and wb_next < NB:
                        for _ in range(4):
                            w_block(wb_next)
                            wb_next += 1
    nc.compile()
    return nc


_NC_CACHE = {}
LAST_EXEC_NS = None


def _get_nc():
    if "full" not in _NC_CACHE:
        _NC_CACHE["full"] = build_nc()
    return _NC_CACHE["full"]


def _run(x, weight, bias, trace=False):
    global LAST_EXEC_NS
    x = np.asarray(x, dtype=np.float32).reshape(T_FULL, I)
    weight = np.asarray(weight, dtype=np.float32)
    bias = np.asarray(bias, dtype=np.float32)

    TT = T_FULL // TSPLIT
    TO = O // OSPLIT
    in_maps = []
    for c in range(N_CORES):
        ti, oj = divmod(c, OSPLIT)
        in_maps.append(
            {
                "x": np.ascontiguousarray(x[ti * TT : (ti + 1) * TT, :]),
                "w": np.ascontiguousarray(weight[oj * TO : (oj + 1) * TO, :]),
                "bias": np.ascontiguousarray(bias[oj * TO : (oj + 1) * TO]),
            }
        )

    nc = _get_nc()
    res = run_bass_kernel_spmd(
        nc, in_maps, core_ids=list(range(N_CORES)), trace=trace
    )
    LAST_EXEC_NS = res.exec_time_ns

    out = np.empty((T_FULL, O), dtype=np.float32)
    for c in range(N_CORES):
        ti, oj = divmod(c, OSPLIT)
        out[ti * TT : (ti + 1) * TT, oj * TO : (oj + 1) * TO] = res.results[c]["out"]
    return out.reshape(B, S, O)


def kernel(x, weight, bias):
    return _run(x, weight, bias, trace=False)


def kernel_traced(x, weight, bias):
    _run(x, weight, bias, trace=True)
    return LAST_EXEC_NS
